# revision 1
# baseline (speedup 1.0000x reference)
"""PhiMoE decoder layer on 8 trn2 NeuronCores.

Fallback v0: pure-jax data-parallel implementation (batch sharded over 8
cores, weights replicated, dense MoE). Used until the Bass kernel lands.
"""
import numpy as np
import jax, jax.numpy as jnp
from jax.sharding import Mesh, PartitionSpec as P
from jax.experimental.shard_map import shard_map

B, L, D, H, KV, HD, E, F = 8, 1024, 1024, 16, 4, 64, 16, 2048
EPS = 1e-5
TOPK = 2
NCORES = 8


def _rmsnorm(x, w, b):
    xf = x.astype(jnp.float32)
    r = jax.lax.rsqrt(jnp.mean(xf * xf, axis=-1, keepdims=True) + EPS)
    return (w.astype(jnp.float32) * (xf * r) + b.astype(jnp.float32)).astype(x.dtype)


def _rope(x, cos, sin):
    d = x.shape[-1]
    x1, x2 = x[..., : d // 2], x[..., d // 2:]
    x_rot = jnp.stack([-x2, x1], axis=-1).reshape(x.shape)
    return x * cos + x_rot * sin


def _forward_shard(x, cos, sin, ln1_w, ln1_b, ln2_w, ln2_b,
                   Wq, bq, Wk, bk, Wv, bv, Wo, bo, gate_w, W1, W2, W3):
    Bx, Lx, Dx = x.shape
    h = _rmsnorm(x, ln1_w, ln1_b)
    q = (h @ Wq + bq).reshape(Bx, Lx, H, HD).transpose(0, 2, 1, 3)
    kk = (h @ Wk + bk).reshape(Bx, Lx, KV, HD).transpose(0, 2, 1, 3)
    vv = (h @ Wv + bv).reshape(Bx, Lx, KV, HD).transpose(0, 2, 1, 3)
    q = _rope(q, cos, sin)
    kk = _rope(kk, cos, sin)
    kk = jnp.repeat(kk, H // KV, axis=1)
    vv = jnp.repeat(vv, H // KV, axis=1)
    scores = jnp.einsum("bhqd,bhkd->bhqk", q, kk) / np.sqrt(HD).astype(np.float32)
    causal = jnp.tril(jnp.ones((Lx, Lx), bool))
    scores = jnp.where(causal, scores, jnp.asarray(-1e30, scores.dtype))
    attn = jax.nn.softmax(scores, axis=-1)
    o = jnp.einsum("bhqk,bhkd->bhqd", attn, vv).transpose(0, 2, 1, 3).reshape(Bx, Lx, H * HD)
    x = x + (o @ Wo + bo)
    h = _rmsnorm(x, ln2_w, ln2_b)
    xf = h.reshape(-1, Dx)
    router_logits = xf @ gate_w
    tw, ti = jax.lax.top_k(router_logits.astype(jnp.float32), TOPK)
    tw = jax.nn.softmax(tw, axis=-1).astype(xf.dtype)
    comb = jnp.sum(jax.nn.one_hot(ti, E, dtype=xf.dtype) * tw[..., None], axis=1)
    out = jnp.zeros_like(xf)
    for e in range(E):
        he = jax.nn.silu(xf @ W1[e]) * (xf @ W3[e])
        out = out + comb[:, e:e + 1] * (he @ W2[e])
    x = x + out.reshape(Bx, Lx, Dx)
    return x, router_logits


def kernel(**inputs):
    devices = jax.devices()[:NCORES]
    mesh = Mesh(np.asarray(devices), ("core",))
    xspec = P("core")
    rspec = P()
    arg_names = ["x", "cos", "sin", "ln1_w", "ln1_b", "ln2_w", "ln2_b",
                 "Wq", "bq", "Wk", "bk", "Wv", "bv", "Wo", "bo",
                 "gate_w", "W1", "W2", "W3"]
    in_specs = tuple(xspec if n == "x" else rspec for n in arg_names)
    fn = shard_map(_forward_shard, mesh=mesh,
                   in_specs=in_specs, out_specs=(P("core"), P("core")),
                   check_rep=False)
    args = [np.asarray(inputs[n]) for n in arg_names]
    xo, lg = jax.jit(fn)(*args)
    return np.asarray(xo), np.asarray(lg)
